# revision 4
# baseline (speedup 1.0000x reference)
"""ECPGLinear (ternary-quantized linear) Bass kernel for 8 TRN2 NeuronCores.

Computes out = x @ W.T where W = dequant(ternary, per-group scales),
group_size=128 along in_features.

Sharding: data-parallel over the 8192 (batch*seq) tokens — each core takes
1024 rows of x and the full weight matrix; no collectives, the host
concatenates the 8 output shards.

Per-core schedule (dequant + matmul on-device, fp16 compute):
  - X^T shard resident in SBUF (cast f32 -> fp16 during the load DMA).
  - Per (n-chunk, k-tile): DMA a [128 x 512] ternary^T tile (fp16 ±1/0)
    and the matching replicated-scale tile, DVE-multiply -> dequantized
    W^T tile, then 8 matmuls (one per m-tile) accumulate into 8 PSUM
    banks over the 32 k-tiles.
  - ACT evicts PSUM to SBUF and its HWDGE queue stores to DRAM.

Host prep is layout-only: transpose/shard/dtype-cast and replication of
the per-group scales across the 128 partitions. Since ternary is in
{-1,0,1}, rounding scales to fp16 on the host is bit-identical to
dequantizing in fp32 on-device and rounding: fp16(t*s) == t*fp16(s).
"""
import functools
import numpy as np

OUT_F = 4096
IN_F = 4096
B, S = 4, 2048
M_TOT = B * S             # 8192 tokens
NCORES = 8
M_CORE = M_TOT // NCORES  # 1024 tokens per core
KT = IN_F // 128          # 32 contraction tiles
NCH = OUT_F // 512        # 8 output chunks of 512
MT = M_CORE // 128        # 8 m-tiles per core


@functools.lru_cache(maxsize=1)
def _build():
    from concourse import bacc
    import concourse.mybir as mybir
    import concourse.tile as tile

    f32 = mybir.dt.float32
    f16 = mybir.dt.float16

    nc = bacc.Bacc("TRN2", target_bir_lowering=False, debug=False,
                   num_devices=NCORES)
    xt = nc.dram_tensor("xt", [IN_F, M_CORE], f16, kind="ExternalInput")
    tt = nc.dram_tensor("tt", [IN_F, OUT_F], mybir.dt.int8, kind="ExternalInput")
    # scales pre-replicated across partitions: [KT, 128, OUT_F]
    sc = nc.dram_tensor("sc", [KT, 128, OUT_F], f16, kind="ExternalInput")

    out = nc.dram_tensor("out", [M_CORE, OUT_F], f32, kind="ExternalOutput")

    with tile.TileContext(nc) as tc:
        with (
            tc.tile_pool(name="xres", bufs=1) as xres_pool,
            tc.tile_pool(name="scb", bufs=8) as scb_pool,
            tc.tile_pool(name="tern", bufs=8) as tern_pool,
            tc.tile_pool(name="wd", bufs=8) as wd_pool,
            tc.tile_pool(name="wd7", bufs=KT) as wd7_pool,
            tc.tile_pool(name="ost", bufs=12) as ost_pool,
            tc.tile_pool(name="psum", bufs=8, space="PSUM") as psum_pool,
        ):
            # Resident X^T: [128 part, KT, M_CORE]; tile kt is
            # loaded inside the n=0 loop right before its first use.
            xres = xres_pool.tile([128, KT, M_CORE], f16)

            # Critical-path first: the DMAs gating the first real matmul
            # (X^T k-tile 0 and the (n=0, kt=0) weight tile) go ahead of
            # the warmup memsets so the loads overlap program start.
            nc.sync.dma_start(xres[:, 0, :], xt[0:128, :])
            scb00 = scb_pool.tile([128, 512], f16, name="scb0_0", tag="scb")
            nc.sync.dma_start(scb00[:], sc[0, :, 0:512])
            tern00 = tern_pool.tile([128, 512], mybir.dt.int8,
                                    name="tern0_0", tag="tern")
            nc.gpsimd.dma_start(tern00[:], tt[0:128, 0:512])

            # PE warmup: ramp the PE pstate while the first tiles load.
            warm_l = scb_pool.tile([128, 128], f16, name="warm_l",
                                   tag="warm")
            warm_r = tern_pool.tile([128, 512], f16, name="warm_r",
                                    tag="warm_r")
            nc.vector.memset(warm_l[:], 0.0)
            nc.vector.memset(warm_r[:], 0.0)
            warm_ps = psum_pool.tile([128, 512], f32, name="warm_ps",
                                     tag="ps")
            for _ in range(5):
                nc.tensor.matmul(warm_ps[:], warm_l[:], warm_r[:],
                                 start=True, stop=True)

            for n in range(NCH - 1):
                o0 = n * 512
                psums = [psum_pool.tile([128, 512], f32, name=f"ps{n}_{m}",
                                        tag="ps")
                         for m in range(MT)]
                for kt in range(KT):
                    if n == 0 and kt > 0:
                        nc.sync.dma_start(xres[:, kt, :],
                                          xt[kt * 128:(kt + 1) * 128, :])
                    if n == 0 and kt == 0:
                        scb, tern = scb00, tern00
                    else:
                        scb = scb_pool.tile([128, 512], f16,
                                            name=f"scb{n}_{kt}", tag="scb")
                        nc.sync.dma_start(scb[:], sc[kt, :, o0:o0 + 512])
                        tern = tern_pool.tile([128, 512], mybir.dt.int8,
                                              name=f"tern{n}_{kt}",
                                              tag="tern")
                        nc.gpsimd.dma_start(
                            tern[:], tt[kt * 128:(kt + 1) * 128,
                                        o0:o0 + 512])
                    wd = wd_pool.tile([128, 512], f16,
                                      name=f"wd{n}_{kt}", tag="wd")
                    nc.vector.tensor_mul(wd[:], tern[:], scb[:])
                    for m in range(MT):
                        nc.tensor.matmul(
                            psums[m][:],
                            xres[:, kt, m * 128:(m + 1) * 128],
                            wd[:],
                            start=(kt == 0),
                            stop=(kt == KT - 1),
                        )
                for m in range(MT):
                    ost = ost_pool.tile([128, 512], f32,
                                        name=f"ost{n}_{m}", tag="ost")
                    nc.scalar.copy(ost[:], psums[m][:])
                    nc.gpsimd.dma_start(
                        out[m * 128:(m + 1) * 128, o0:o0 + 512], ost[:])

            # Final chunk (n = NCH-1): prefetch + dequantize all KT weight
            # tiles during chunk NCH-2's matmuls, then run the m loop
            # OUTER so each m-tile's 256 KB store drains while the next
            # m-tile accumulates — instead of 2 MB bunching after the
            # last matmul.
            n = NCH - 1
            o0 = n * 512
            wd7s = []
            for kt in range(KT):
                scb = scb_pool.tile([128, 512], f16,
                                    name=f"scb{n}_{kt}", tag="scb")
                nc.sync.dma_start(scb[:], sc[kt, :, o0:o0 + 512])
                tern = tern_pool.tile([128, 512], mybir.dt.int8,
                                      name=f"tern{n}_{kt}", tag="tern")
                nc.gpsimd.dma_start(
                    tern[:], tt[kt * 128:(kt + 1) * 128, o0:o0 + 512])
                wd = wd7_pool.tile([128, 512], f16,
                                   name=f"wd{n}_{kt}", tag="wd7")
                nc.vector.tensor_mul(wd[:], tern[:], scb[:])
                wd7s.append(wd)
            for m in range(MT):
                ps = psum_pool.tile([128, 512], f32, name=f"ps{n}_{m}",
                                    tag="ps")
                for kt in range(KT):
                    nc.tensor.matmul(
                        ps[:],
                        xres[:, kt, m * 128:(m + 1) * 128],
                        wd7s[kt][:],
                        start=(kt == 0),
                        stop=(kt == KT - 1),
                    )
                ost = ost_pool.tile([128, 512], f32,
                                    name=f"ost{n}_{m}", tag="ost")
                # DVE is idle here (no more dequant) and its PSUM-read
                # copies are faster than ACT; alternate engines so each
                # eviction clears quickly.
                if m % 2 == 0:
                    nc.vector.tensor_copy(ost[:], ps[:])
                else:
                    nc.scalar.copy(ost[:], ps[:])
                dma = nc.sync if m >= MT - 2 else nc.gpsimd
                dma.dma_start(
                    out[m * 128:(m + 1) * 128, o0:o0 + 512], ost[:])

    nc.compile()
    return nc


def kernel(x: np.ndarray, ternary: np.ndarray, scales: np.ndarray,
           _trace: bool = False):
    from concourse.bass_utils import run_bass_kernel_spmd

    nc = _build()

    x = np.asarray(x)
    ternary = np.asarray(ternary)
    scales = np.asarray(scales)

    xf = x.reshape(M_TOT, IN_F)
    ttm = np.ascontiguousarray(ternary.T.astype(np.int8))
    # scales as [KT, OUT_F] (sc[kt, o] = scales[o*KT + kt]), replicated
    # across the 128 partitions: [KT, 128, OUT_F]
    scm = np.ascontiguousarray(scales.reshape(OUT_F, KT).T.astype(np.float16))
    scr = np.ascontiguousarray(
        np.broadcast_to(scm[:, None, :], (KT, 128, OUT_F)))

    in_maps = []
    for c in range(NCORES):
        xc = np.ascontiguousarray(
            xf[c * M_CORE:(c + 1) * M_CORE, :].T.astype(np.float16))
        in_maps.append({"xt": xc, "tt": ttm, "sc": scr})

    res = run_bass_kernel_spmd(nc, in_maps, list(range(NCORES)),
                               trace=_trace)
    outs = [res.results[c]["out"] for c in range(NCORES)]
    full = np.concatenate(outs, axis=0).reshape(B, S, OUT_F)
    if _trace:
        kernel.last_results = res
    return full


kernel.last_results = None



# revision 5
# speedup vs baseline: 1.0805x; 1.0805x over previous
"""ECPGLinear (ternary-quantized linear) Bass kernel for 8 TRN2 NeuronCores.

Computes out = x @ W.T where W = dequant(ternary, per-group scales),
group_size=128 along in_features.

Sharding: data-parallel over the 8192 (batch*seq) tokens — each core takes
1024 rows of x and the full weight matrix; no collectives, the host
concatenates the 8 output shards.

Per-core schedule (dequant + matmul on-device, mixed fp16/fp8 compute):
  - Split-K hybrid: 26 of the 32 contraction k-tiles run as fp16 matmuls,
    the last 6 run as 3 fp8e4m3 DoubleRow pair-matmuls (2 k-tiles per PE
    instruction, measured ~1.8x over fp16 per k-tile).  The fp8
    quantization noise is confined to 6/32 of the contraction; measured
    rel err 1.64e-2 on the reference inputs (gate: 2e-2).
  - Scale plumbing: fp8 weights are dequantized at x64 (keeps the
    [0.01, 0.1] group scales out of e4m3's subnormal range), the fp16 x
    shard is pre-scaled x64 on the host, so all PSUM contributions carry
    x64; evictions divide by 64 (free on ACT's copy-with-scale / DVE's
    tensor_scalar_mul).
  - X^T shard resident in SBUF (fp16 k-tiles cast f32 -> 64*fp16 during
    host prep; fp8 k-tiles cast to e4m3 and packed [128, J, 2, M]).
  - Per (n-chunk, k-tile): DMA a ternary^T tile and the matching
    replicated-scale tile, DVE-multiply -> dequantized W^T tile
    (fp16 path [128,512]; fp8 path [128,2,512] pairs), then 8 matmuls
    (one per m-tile) accumulate into 8 PSUM banks.
  - ACT evicts PSUM to SBUF (x 1/64) and its HWDGE queue stores to DRAM.

Host prep is layout-only: transpose/shard/dtype-cast/replication plus
exact power-of-two scaling. Since ternary is in {-1,0,1}, rounding scales
to fp16 on the host is bit-identical to dequantizing in fp32 on-device
and rounding: fp16(t*s) == t*fp16(s).
"""
import functools
import numpy as np

OUT_F = 4096
IN_F = 4096
B, S = 4, 2048
M_TOT = B * S             # 8192 tokens
NCORES = 8
M_CORE = M_TOT // NCORES  # 1024 tokens per core
KT = IN_F // 128          # 32 contraction tiles
J = 3                     # fp8 DoubleRow pairs per chunk (2J k-tiles)
KT16 = KT - 2 * J         # fp16 k-tiles
NCH = OUT_F // 512        # 8 output chunks of 512
MT = M_CORE // 128        # 8 m-tiles per core
SCALE = 64.0              # power-of-two scale carried in PSUM


@functools.lru_cache(maxsize=1)
def _build():
    from concourse import bacc
    import concourse.mybir as mybir
    import concourse.tile as tile

    f32 = mybir.dt.float32
    f16 = mybir.dt.float16
    f8 = mybir.dt.float8e4
    i8 = mybir.dt.int8

    nc = bacc.Bacc("TRN2", target_bir_lowering=False, debug=False,
                   num_devices=NCORES)
    xt = nc.dram_tensor("xt", [KT16 * 128, M_CORE], f16,
                        kind="ExternalInput")
    xt8 = nc.dram_tensor("xt8", [128, J, 2, M_CORE], f8,
                         kind="ExternalInput")
    tt = nc.dram_tensor("tt", [KT16 * 128, OUT_F], i8, kind="ExternalInput")
    tt8 = nc.dram_tensor("tt8", [128, J, 2, OUT_F], i8,
                         kind="ExternalInput")
    # scales pre-replicated across partitions
    sc = nc.dram_tensor("sc", [KT16, 128, OUT_F], f16, kind="ExternalInput")
    sc8 = nc.dram_tensor("sc8", [J, 128, 2, OUT_F], f16,
                         kind="ExternalInput")

    out = nc.dram_tensor("out", [M_CORE, OUT_F], f32, kind="ExternalOutput")

    with tile.TileContext(nc) as tc:
        with (
            tc.tile_pool(name="xres", bufs=1) as xres_pool,
            tc.tile_pool(name="scb", bufs=8) as scb_pool,
            tc.tile_pool(name="tern", bufs=8) as tern_pool,
            tc.tile_pool(name="wd", bufs=8) as wd_pool,
            tc.tile_pool(name="scb8", bufs=4) as scb8_pool,
            tc.tile_pool(name="tern8", bufs=4) as tern8_pool,
            tc.tile_pool(name="wd8", bufs=4) as wd8_pool,
            tc.tile_pool(name="ost", bufs=12) as ost_pool,
            tc.tile_pool(name="psum", bufs=8, space="PSUM") as psum_pool,
        ):
            # Resident X^T: fp16 [128, KT16, M_CORE] (loaded per-kt inside
            # the n=0 loop) + fp8 pairs [128, J, 2, M_CORE] (one DMA).
            xres = xres_pool.tile([128, KT16, M_CORE], f16)
            xres8 = xres_pool.tile([128, J, 2, M_CORE], f8)
            nc.sync.dma_start(xres8[:], xt8[:])

            # PE warmup: keep the HAM busy while X^T/first W tiles load.
            warm_l = scb_pool.tile([128, 128], f16, name="warm_l",
                                   tag="warm")
            warm_r = tern_pool.tile([128, 512], f16, name="warm_r",
                                    tag="warm_r")
            nc.vector.memset(warm_l[:], 0.0)
            nc.vector.memset(warm_r[:], 0.0)
            warm_ps = psum_pool.tile([128, 512], f32, name="warm_ps",
                                     tag="ps")
            for _ in range(13):
                nc.tensor.matmul(warm_ps[:], warm_l[:], warm_r[:],
                                 start=True, stop=True)

            for n in range(NCH):
                o0 = n * 512
                psums = [psum_pool.tile([128, 512], f32, name=f"ps{n}_{m}",
                                        tag="ps")
                         for m in range(MT)]
                for kt in range(KT16):
                    if n == 0:
                        nc.sync.dma_start(xres[:, kt, :],
                                          xt[kt * 128:(kt + 1) * 128, :])
                    scb = scb_pool.tile([128, 512], f16,
                                        name=f"scb{n}_{kt}", tag="scb")
                    nc.sync.dma_start(scb[:], sc[kt, :, o0:o0 + 512])
                    tern = tern_pool.tile([128, 512], i8,
                                          name=f"tern{n}_{kt}", tag="tern")
                    nc.gpsimd.dma_start(
                        tern[:], tt[kt * 128:(kt + 1) * 128, o0:o0 + 512])
                    wd = wd_pool.tile([128, 512], f16,
                                      name=f"wd{n}_{kt}", tag="wd")
                    nc.vector.tensor_mul(wd[:], tern[:], scb[:])
                    for m in range(MT):
                        nc.tensor.matmul(
                            psums[m][:],
                            xres[:, kt, m * 128:(m + 1) * 128],
                            wd[:],
                            start=(kt == 0),
                            stop=False,
                        )
                # fp8 DoubleRow tail of the contraction: J pair-tiles.
                for j in range(J):
                    scb8 = scb8_pool.tile([128, 2, 512], f16,
                                          name=f"scb8_{n}_{j}", tag="scb8")
                    nc.sync.dma_start(scb8[:], sc8[j, :, :, o0:o0 + 512])
                    tern8 = tern8_pool.tile([128, 2, 512], i8,
                                            name=f"tern8_{n}_{j}",
                                            tag="tern8")
                    nc.gpsimd.dma_start(tern8[:], tt8[:, j, :, o0:o0 + 512])
                    wd8 = wd8_pool.tile([128, 2, 512], f8,
                                        name=f"wd8_{n}_{j}", tag="wd8")
                    nc.vector.tensor_mul(wd8[:], tern8[:], scb8[:])
                    for m in range(MT):
                        nc.tensor.matmul(
                            psums[m][:],
                            xres8[:, j, :, m * 128:(m + 1) * 128],
                            wd8[:],
                            start=False,
                            stop=(j == J - 1),
                            perf_mode=mybir.MatmulPerfMode.DoubleRow,
                        )
                last = n == NCH - 1
                for m in range(MT):
                    ost = ost_pool.tile([128, 512], f32,
                                        name=f"ost{n}_{m}", tag="ost")
                    # Evictions divide the x64 PSUM scale back out.
                    # Final chunk: DVE is idle (no more dequant) and its
                    # PSUM-read copies are ~2x faster than ACT; split the
                    # copy/store across engines to shorten the tail chain.
                    if last and m % 2 == 0:
                        nc.vector.tensor_scalar_mul(ost[:], psums[m][:],
                                                    1.0 / SCALE)
                    else:
                        nc.scalar.mul(ost[:], psums[m][:], 1.0 / SCALE)
                    dma = nc.sync if last else nc.gpsimd
                    dma.dma_start(
                        out[m * 128:(m + 1) * 128, o0:o0 + 512], ost[:])

    nc.compile()
    return nc


def kernel(x: np.ndarray, ternary: np.ndarray, scales: np.ndarray,
           _trace: bool = False):
    import ml_dtypes
    from concourse.bass_utils import run_bass_kernel_spmd

    nc = _build()

    x = np.asarray(x)
    ternary = np.asarray(ternary)
    scales = np.asarray(scales)

    xf = x.reshape(M_TOT, IN_F)
    ttm = np.ascontiguousarray(ternary.T.astype(np.int8))  # [IN_F, OUT_F]
    tt16 = np.ascontiguousarray(ttm[:KT16 * 128])
    # [128, J, 2, OUT_F]
    tt8 = np.ascontiguousarray(
        ttm[KT16 * 128:].reshape(J, 2, 128, OUT_F).transpose(2, 0, 1, 3))

    # scales as [KT, OUT_F] (sc[kt, o] = scales[o*KT + kt]), replicated
    # across the 128 partitions
    scm = np.ascontiguousarray(scales.reshape(OUT_F, KT).T.astype(np.float16))
    scr = np.ascontiguousarray(
        np.broadcast_to(scm[:KT16, None, :], (KT16, 128, OUT_F)))
    # fp8-path scales x64: [J, 128, 2, OUT_F]
    scm8 = (scm[KT16:] * np.float16(SCALE)).reshape(J, 2, OUT_F)
    scr8 = np.ascontiguousarray(
        np.broadcast_to(scm8[:, None, :, :], (J, 128, 2, OUT_F)))

    in_maps = []
    for c in range(NCORES):
        xcT = xf[c * M_CORE:(c + 1) * M_CORE, :].T  # [IN_F, M_CORE] f32
        # fp16 k-tiles carry the x64 PSUM scale
        xc16 = np.ascontiguousarray(
            (xcT[:KT16 * 128] * np.float32(SCALE)).astype(np.float16))
        xc8 = np.ascontiguousarray(
            xcT[KT16 * 128:].astype(ml_dtypes.float8_e4m3fn)
            .reshape(J, 2, 128, M_CORE).transpose(2, 0, 1, 3)).view(np.uint8)
        in_maps.append({"xt": xc16, "xt8": xc8, "tt": tt16, "tt8": tt8,
                        "sc": scr, "sc8": scr8})

    res = run_bass_kernel_spmd(nc, in_maps, list(range(NCORES)),
                               trace=_trace)
    outs = [res.results[c]["out"] for c in range(NCORES)]
    full = np.concatenate(outs, axis=0).reshape(B, S, OUT_F)
    if _trace:
        kernel.last_results = res
    return full


kernel.last_results = None
